# revision 10
# baseline (speedup 1.0000x reference)
"""Trainium2 Bass kernel for nn_BilinearAttention.

Reference computation (per batch row b):
    Q = q @ Wq; K = X @ Wk; V = X @ Wv; QU = Q @ U
    scores = QU . K^T           (B, M)
    attn   = softmax(mask(scores))
    ctx    = attn @ V           (B, d_attn)
    fused  = ctx @ Wo + bo + q

Key algebraic restructuring (avoids ever materialising K or V, which are the
only O(B*M*d_t*d_a) matmuls):
    R      = QU @ Wk^T                  (B, d_t)   tiny
    scores = sum_t X[b,m,t] * R[b,t]    fused mult+reduce on VectorE
    ctx_t  = sum_m attn[b,m] * X[b,m,:] N=1 matmuls on TensorE (contract m)
    ctx    = ctx_t @ Wv; fused = ctx @ Wo + bo + q

X (desc_tokens) is streamed from HBM exactly once, in its natural
(m-partition, t-free) layout. Sharding: pure data parallel, batch/8 per core.
"""

import os
from contextlib import ExitStack

import numpy as np

B, M, DQ, DT, DA = 2048, 200, 512, 768, 64
NCORES = 8
BL = B // NCORES  # 256 batch rows per core
MC0 = 128         # m-chunk 0 partitions
MC1 = M - MC0     # m-chunk 1 partitions (72)
F32 = None        # set lazily (mybir import)


def build(nc, bl=BL, g=8):
    """Emit the per-core kernel program into `nc`. bl = local batch, g = group size."""
    import concourse.tile as tile
    import concourse.mybir as mybir
    from concourse.masks import make_identity

    f32 = mybir.dt.float32
    i32 = mybir.dt.int32
    MULT = mybir.AluOpType.mult
    ADD = mybir.AluOpType.add
    EXP = mybir.ActivationFunctionType.Exp

    ng = bl // g
    nbc = (bl + 127) // 128          # batch chunks of 128 (2 for bl=256)
    bc_sizes = [min(128, bl - 128 * c) for c in range(nbc)]

    # ---- DRAM tensors (names must match setup_inputs keys) ----
    qv_d = nc.dram_tensor("q_vec", (bl, DQ), f32, kind="ExternalInput").ap()
    x_d = nc.dram_tensor("desc_tokens", (bl, M, DT), f32, kind="ExternalInput").ap()
    mask_d = nc.dram_tensor("desc_mask", (bl, M), i32, kind="ExternalInput").ap()
    wq_d = nc.dram_tensor("Wq", (DQ, DA), f32, kind="ExternalInput").ap()
    wk_d = nc.dram_tensor("Wk", (DT, DA), f32, kind="ExternalInput").ap()
    wv_d = nc.dram_tensor("Wv", (DT, DA), f32, kind="ExternalInput").ap()
    u_d = nc.dram_tensor("U", (DA, DA), f32, kind="ExternalInput").ap()
    wo_d = nc.dram_tensor("Wo", (DA, DQ), f32, kind="ExternalInput").ap()
    bo_d = nc.dram_tensor("bo", (DQ,), f32, kind="ExternalInput").ap()
    fused_d = nc.dram_tensor("fused", (bl, DQ), f32, kind="ExternalOutput").ap()
    attn_d = nc.dram_tensor("attn", (bl, M), f32, kind="ExternalOutput").ap()

    with tile.TileContext(nc) as tc, ExitStack() as ctx:
        const = ctx.enter_context(tc.tile_pool(name="const", bufs=1))

        ident = const.tile([128, 128], f32)
        make_identity(nc, ident[:])
        ones_col = const.tile([128, 1], f32)
        nc.gpsimd.memset(ones_col[:], 1.0)
        ones_row = const.tile([1, 128], f32)
        nc.gpsimd.memset(ones_row[:], 1.0)

        # ---- load weights ----
        wq_sb = const.tile([128, DQ // 128, DA], f32)
        nc.sync.dma_start(wq_sb[:], wq_d.rearrange("(o p) d -> p o d", p=128))
        wk_sb = const.tile([128, DT // 128, DA], f32)
        nc.sync.dma_start(wk_sb[:], wk_d.rearrange("(o p) d -> p o d", p=128))
        wv_sb = const.tile([128, DT // 128, DA], f32)
        nc.sync.dma_start(wv_sb[:], wv_d.rearrange("(o p) d -> p o d", p=128))
        u_sb = const.tile([DA, DA], f32)
        nc.sync.dma_start(u_sb[:], u_d)
        wo_sb = const.tile([DA, DQ], f32)
        nc.sync.dma_start(wo_sb[:], wo_d)
        bo_sb = const.tile([128, DQ // 128], f32)
        with nc.allow_non_contiguous_dma(reason="tiny bias vector, one-time"):
            nc.sync.dma_start(bo_sb[:], bo_d.rearrange("(o p) -> p o", p=128))

        # persistent score-space tiles
        scoresT0 = const.tile([128, bl], f32)   # (m 0:128, b)
        scoresT1 = const.tile([MC1, bl], f32)   # (m 128:200, b)
        expTm0 = const.tile([128, bl], f32)
        expTm1 = const.tile([MC1, bl], f32)
        maskT0 = const.tile([128, bl], f32)
        maskT1 = const.tile([MC1, bl], f32)
        qvT_sb = const.tile([128, DQ // 128, bl], f32)
        scratch = const.tile([128, DT], f32)    # TTR throwaway product
        dram_pool = ctx.enter_context(tc.tile_pool(name="dram", bufs=1, space="DRAM"))
        r_dram = dram_pool.tile([bl, DT], f32)  # R rows staged for partition-0 reload

        # ---- head phase: transposes + small matmuls ----
        with tc.tile_pool(name="head_sb", bufs=2) as head_sb, \
             tc.tile_pool(name="head_ps", bufs=2, space="PSUM") as head_ps, \
             tc.tile_pool(name="head_ps2", bufs=1, space="PSUM") as head_ps2:
            # q_vec native, then PE-transpose to qvT (dq-part, b)
            qv_sb = head_sb.tile([128, nbc, DQ], f32, tag="qv")
            nc.sync.dma_start(qv_sb[:], qv_d.rearrange("(c p) q -> p c q", p=128))
            for c in range(nbc):
                for o in range(DQ // 128):
                    ps = head_ps.tile([128, 128], f32, tag="hp")
                    nc.tensor.transpose(ps[:], qv_sb[:, c, o * 128:(o + 1) * 128], ident[:])
                    nc.scalar.copy(qvT_sb[:, o, c * 128:(c + 1) * 128], ps[:])

            # Wk^T (da, t)
            wkT_sb = const.tile([DA, DT], f32)
            for c in range(DT // 128):
                ps = head_ps.tile([DA, 128], f32, tag="hp")
                nc.tensor.transpose(ps[:], wk_sb[:, c, :], ident[:])
                nc.scalar.copy(wkT_sb[:, c * 128:(c + 1) * 128], ps[:])

            # mask: load native int32, cast to f32, transpose to (m, b)
            mask_i = head_sb.tile([128, nbc, M], i32, tag="mi")
            nc.sync.dma_start(mask_i[:], mask_d.rearrange("(c p) m -> p c m", p=128))
            mask_f = head_sb.tile([128, nbc, M], f32, tag="mf")
            nc.vector.tensor_copy(mask_f[:], mask_i[:])
            for c in range(nbc):
                ps = head_ps.tile([128, 128], f32, tag="hp")
                nc.tensor.transpose(ps[:], mask_f[:, c, 0:128], ident[:])
                nc.scalar.copy(maskT0[:, c * 128:(c + 1) * 128], ps[:])
                ps2 = head_ps.tile([MC1, 128], f32, tag="hp2")
                nc.tensor.transpose(ps2[:], mask_f[:, c, 128:M], ident[:])
                nc.scalar.copy(maskT1[:, c * 128:(c + 1) * 128], ps2[:])

            # Q^T = Wq^T @ qv^T  (da, bl)
            qT_ps = head_ps.tile([DA, bl], f32, tag="hp")
            for o in range(DQ // 128):
                nc.tensor.matmul(qT_ps[:], wq_sb[:, o, :], qvT_sb[:, o, :],
                                 start=(o == 0), stop=(o == DQ // 128 - 1))
            qT_sb = head_sb.tile([DA, bl], f32, tag="qts")
            nc.scalar.copy(qT_sb[:], qT_ps[:])

            # QU^T = U^T @ Q^T
            quT_ps = head_ps.tile([DA, bl], f32, tag="hp")
            nc.tensor.matmul(quT_ps[:], u_sb[:], qT_sb[:])
            quT_sb = head_sb.tile([DA, bl], f32, tag="quts")
            nc.scalar.copy(quT_sb[:], quT_ps[:])

            # R = QU @ Wk^T  (b-part, t), staged out to DRAM for per-row reload
            for c in range(nbc):
                bs = bc_sizes[c]
                r_ps = head_ps2.tile([128, 1024], f32, tag="rps")
                nc.tensor.matmul(r_ps[:bs, 0:512], quT_sb[:, c * 128:c * 128 + bs],
                                 wkT_sb[:, 0:512])
                nc.tensor.matmul(r_ps[:bs, 512:DT], quT_sb[:, c * 128:c * 128 + bs],
                                 wkT_sb[:, 512:DT])
                r_sb = head_sb.tile([128, DT], f32, tag="rsb")
                nc.scalar.copy(r_sb[:bs, :], r_ps[:bs, 0:DT])
                nc.sync.dma_start(r_dram[c * 128:c * 128 + bs, :], r_sb[:bs, :])

        # ---- main loop over groups of g batch rows ----
        ctx_pool = ctx.enter_context(tc.tile_pool(name="ctxps", bufs=1, space="PSUM"))
        loop_ctx = ctx.enter_context(ExitStack())
        xp0 = loop_ctx.enter_context(tc.tile_pool(name="x0", bufs=3))
        xp1 = loop_ctx.enter_context(tc.tile_pool(name="x1", bufs=3))
        rstage_pool = loop_ctx.enter_context(tc.tile_pool(name="rstage", bufs=8))
        rrep_pool = loop_ctx.enter_context(tc.tile_pool(name="rrep", bufs=2, space="PSUM"))
        # ctx^T accumulators: 3 banks, each holding two 128-row t-chunks x bl cols
        ctx_ps = [ctx_pool.tile([128, 2 * bl], f32, name=f"ctxps{j}") for j in range(3)]

        def emit_context(g_idx, xt0_, xt1_):
            for j in range(g):
                b = g_idx * g + j
                for t2 in range(DT // 128):
                    dst = ctx_ps[t2 // 2][:, (t2 % 2) * bl + b:(t2 % 2) * bl + b + 1]
                    nc.tensor.matmul(dst, xt0_[:, j, t2 * 128:(t2 + 1) * 128],
                                     expTm0[:, b:b + 1], start=True, stop=False)
                    nc.tensor.matmul(dst, xt1_[:, j, t2 * 128:(t2 + 1) * 128],
                                     expTm1[:, b:b + 1], start=False, stop=True)

        prev = None  # (g_idx, xt0, xt1) pending context emission
        for gi in range(ng):
            gsl = slice(gi * g, (gi + 1) * g)
            xt0 = xp0.tile([128, g, DT], f32)
            nc.sync.dma_start(xt0[:], x_d[gsl, 0:128, :].rearrange("b m t -> m b t"))
            xt1 = xp1.tile([MC1, g, DT], f32)
            nc.sync.dma_start(xt1[:], x_d[gsl, 128:M, :].rearrange("b m t -> m b t"))

            for j in range(g):
                b = gi * g + j
                rstage = rstage_pool.tile([1, DT], f32)
                nc.sync.dma_start(rstage[:], r_dram[b:b + 1, :])
                rrep = rrep_pool.tile([128, 1024], f32)
                nc.tensor.matmul(rrep[:, 0:512], ones_row[:], rstage[:, 0:512])
                nc.tensor.matmul(rrep[:, 512:DT], ones_row[:], rstage[:, 512:DT])
                nc.vector.affine_mul_reduce(
                    out=scratch[:], accum_out=scoresT0[:, b:b + 1],
                    in0=xt0[:, j, :], in1=rrep[:, 0:DT], scale=1.0, bias=0.0)
                nc.vector.affine_mul_reduce(
                    out=scratch[:MC1, :], accum_out=scoresT1[:, b:b + 1],
                    in0=xt1[:, j, :], in1=rrep[:MC1, 0:DT], scale=1.0, bias=0.0)

            # exp then mask (exp of true scores is finite; masked lanes zeroed after)
            nc.scalar.activation(expTm0[:, gsl], scoresT0[:, gsl], EXP)
            nc.scalar.activation(expTm1[:, gsl], scoresT1[:, gsl], EXP)
            nc.vector.tensor_mul(expTm0[:, gsl], expTm0[:, gsl], maskT0[:, gsl])
            nc.vector.tensor_mul(expTm1[:, gsl], expTm1[:, gsl], maskT1[:, gsl])

            if prev is not None:
                emit_context(*prev)
            prev = (gi, xt0, xt1)
        emit_context(*prev)
        loop_ctx.close()

        # ---- epilogue: softmax denominators, attn output, projections ----
        with tc.tile_pool(name="end_sb", bufs=2) as end_sb, \
             tc.tile_pool(name="end_ps", bufs=2, space="PSUM") as end_ps, \
             tc.tile_pool(name="end_ps1", bufs=1, space="PSUM") as end_ps1:
            denom_ps = end_ps1.tile([1, bl], f32, name="denom")
            nc.tensor.matmul(denom_ps[:], ones_col[:], expTm0[:], start=True, stop=False)
            nc.tensor.matmul(denom_ps[:], ones_col[:MC1, :], expTm1[:], start=False, stop=True)
            denom_sb = end_sb.tile([1, bl], f32, tag="den")
            nc.scalar.copy(denom_sb[:], denom_ps[:])
            recip_sb = end_sb.tile([1, bl], f32, tag="rec")
            nc.vector.reciprocal(recip_sb[:], denom_sb[:])
            rrepc_ps = end_ps1.tile([128, bl], f32, name="rrepc")
            nc.tensor.matmul(rrepc_ps[:], ones_row[:], recip_sb[:])
            rrepc_sb = end_sb.tile([128, bl], f32, tag="recrep")
            nc.scalar.copy(rrepc_sb[:], rrepc_ps[:])

            # attn output: attnT = expTm * recip, transpose back, store
            attnT0 = end_sb.tile([128, bl], f32, tag="at0")
            attnT1 = end_sb.tile([MC1, bl], f32, tag="at1")
            nc.vector.tensor_mul(attnT0[:], expTm0[:], rrepc_sb[:])
            nc.vector.tensor_mul(attnT1[:], expTm1[:], rrepc_sb[:MC1, :])
            attn_sb = end_sb.tile([128, nbc, M], f32, tag="attn")
            for c in range(nbc):
                ps = end_ps.tile([128, 128], f32, tag="ep")
                nc.tensor.transpose(ps[:], attnT0[:, c * 128:(c + 1) * 128], ident[:])
                nc.scalar.copy(attn_sb[:, c, 0:128], ps[:])
                ps2 = end_ps.tile([128, MC1], f32, tag="ep")
                nc.tensor.transpose(ps2[:], attnT1[:, c * 128:(c + 1) * 128],
                                    ident[:MC1, :MC1])
                nc.scalar.copy(attn_sb[:, c, 128:M], ps2[:])
            nc.sync.dma_start(attn_d.rearrange("(c p) m -> p c m", p=128), attn_sb[:])

            # ctx^T scaled by 1/denom, then context^T = Wv^T @ ctx_t^T
            ctxT_sb = [end_sb.tile([128, bl], f32, tag=f"ctxT{t2}", name=f"ctxT{t2}")
                       for t2 in range(DT // 128)]
            for t2 in range(DT // 128):
                nc.vector.tensor_mul(
                    ctxT_sb[t2][:],
                    ctx_ps[t2 // 2][:, (t2 % 2) * bl:(t2 % 2 + 1) * bl],
                    rrepc_sb[:])
            ctxa_ps = end_ps1.tile([DA, bl], f32, name="ctxa")
            for t2 in range(DT // 128):
                nc.tensor.matmul(ctxa_ps[:], wv_sb[:, t2, :], ctxT_sb[t2][:],
                                 start=(t2 == 0), stop=(t2 == DT // 128 - 1))
            ctxa_sb = end_sb.tile([DA, bl], f32, tag="ctxa")
            nc.scalar.copy(ctxa_sb[:], ctxa_ps[:])

            # fused^T = Wo^T @ context^T + bo + qv^T, then transpose + store
            fused_sb = end_sb.tile([128, nbc, DQ], f32, tag="fused")
            for i in range(DQ // 128):
                fT_ps = end_ps.tile([128, bl], f32, tag="ep")
                nc.tensor.matmul(fT_ps[:], wo_sb[:, i * 128:(i + 1) * 128], ctxa_sb[:])
                fT_sb = end_sb.tile([128, bl], f32, tag="fts")
                nc.vector.scalar_tensor_tensor(
                    out=fT_sb[:], in0=fT_ps[:], scalar=bo_sb[:, i:i + 1],
                    in1=qvT_sb[:, i, :], op0=ADD, op1=ADD)
                for c in range(nbc):
                    ps = end_ps.tile([128, 128], f32, tag="ep")
                    nc.tensor.transpose(ps[:], fT_sb[:, c * 128:(c + 1) * 128], ident[:])
                    nc.scalar.copy(fused_sb[:, c, i * 128:(i + 1) * 128], ps[:])
            nc.sync.dma_start(fused_d.rearrange("(c p) q -> p c q", p=128), fused_sb[:])

    return nc


def _make_nc():
    from concourse import bacc
    return bacc.Bacc("TRN2", target_bir_lowering=False, debug=False)


_COMPILED = {}


def _get_compiled(bl, g):
    key = (bl, g)
    if key not in _COMPILED:
        nc = _make_nc()
        build(nc, bl=bl, g=g)
        nc.compile()
        _COMPILED[key] = nc
    return _COMPILED[key]


def kernel(q_vec, desc_tokens, desc_mask, Wq, Wk, Wv, U, Wo, bo, trace=False):
    """Full-input, full-output entry point. Shards batch across 8 cores."""
    from concourse.bass_utils import run_bass_kernel_spmd

    q_vec = np.ascontiguousarray(q_vec, dtype=np.float32)
    desc_tokens = np.ascontiguousarray(desc_tokens, dtype=np.float32)
    desc_mask = np.ascontiguousarray(desc_mask, dtype=np.int32)
    weights = {
        "Wq": np.ascontiguousarray(Wq, dtype=np.float32),
        "Wk": np.ascontiguousarray(Wk, dtype=np.float32),
        "Wv": np.ascontiguousarray(Wv, dtype=np.float32),
        "U": np.ascontiguousarray(U, dtype=np.float32),
        "Wo": np.ascontiguousarray(Wo, dtype=np.float32),
        "bo": np.ascontiguousarray(bo, dtype=np.float32),
    }

    bl = q_vec.shape[0] // NCORES
    nc = _get_compiled(bl, 8)

    in_maps = []
    for i in range(NCORES):
        s = slice(i * bl, (i + 1) * bl)
        m = {"q_vec": q_vec[s], "desc_tokens": desc_tokens[s], "desc_mask": desc_mask[s]}
        m.update(weights)
        in_maps.append(m)

    res = run_bass_kernel_spmd(nc, in_maps, core_ids=list(range(NCORES)), trace=trace)
    fused = np.concatenate([r["fused"] for r in res.results], axis=0)
    attn = np.concatenate([r["attn"] for r in res.results], axis=0)
    kernel.last_results = res
    return fused, attn
